# revision 59
# baseline (speedup 1.0000x reference)
"""Multi-head self-attention (B=4, S=2048, D=2048, H=16, hd=128) on 8 trn2
NeuronCores.

Sharding: tensor-parallel over heads. Core c owns heads {2c, 2c+1}:
  - computes q/k/v projections for its 2 heads over all tokens,
  - runs causal attention for its (4 batches x 2 heads) units,
  - computes a partial output projection with its 256 rows of Wo.
Host sums the 8 partial outputs and adds bo.

On-chip layouts keep activations transposed ([feature, token]) so no
transposes are needed anywhere except V (PE-transpose per 128x128 block):
  qT/kT: [j, t] from W-stationary matmuls (lhsT = W tile, rhs = xT tile)
  S^T:   [t_k, t_q] tiles (lhsT = kT tile, rhs = qT chunk); softmax runs
         along the partition axis: exp on ACT (no max subtraction --
         weights are scaled 0.02, logits are O(1)); the exp-sum G
         accumulates in two partial chains (DVE + GPSIMD), is summed and
         broadcast across partitions by one GPSIMD partition_all_reduce,
         inverted in place (DVE reciprocal), and applied by one DVE
         multiply.
  U^T:   [j, t_q] = accumulated (lhsT = V tile [t_k, j], rhs = exp(S^T)).
  O^T:   [d', t] partial = (lhsT = Wo tile [j, d'], rhs = Yn^T).
Causality: only lower-triangle key-tiles are computed; the 4 diagonal
128x512 tile positions use precomputed 0/1 masks (multiplied after exp).

Scheduling shape (per batch): projections -> V transposes -> attention
units (h, c), with the previous unit's softmax normalization emitted at
the start of the next unit and the PREVIOUS batch's output-projection
tile-groups interleaved through the ACT-paced attention stream so the
tensor engine always has independent work.
"""

import math

import numpy as np
import ml_dtypes

import concourse.bass as bass
import concourse.bacc as bacc
import concourse.mybir as mybir
import concourse.tile as tile
from concourse.masks import make_identity
from concourse.bass_utils import run_bass_kernel_spmd

BF16 = mybir.dt.bfloat16
F16 = mybir.dt.float16
F32 = mybir.dt.float32
F8 = mybir.dt.float8e4

B, S, D_MODEL = 4, 2048, 2048
N_HEADS, HEAD_DIM = 16, 128
N_CORES = 8
H_PER = N_HEADS // N_CORES          # 2 heads per core
JL = H_PER * HEAD_DIM               # 256 local j-columns per of q/k/v
T = B * S                           # 8192 tokens
KD = D_MODEL // 128                 # 16 contraction tiles over d_model
TC = S // 512                       # 4 token chunks of 512 per batch
NJM = 3 * H_PER                     # 6 output j-tiles for fused qkv
SCALE = 1.0 / math.sqrt(HEAD_DIM)

# split-fp8 projection: x is pre-scaled by SX and split into hi+lo fp8
# k-slots; w by SW. q/k/v come out scaled by SX*SW = 256; scores carry
# 256^2, folded into the exp scale; outT carries 256, divided on host.
SX = 8.0
SW = 32.0
PSCALE = SX * SW                    # 256
NDR = 3 * KD // 2                   # 24 DoubleRow matmuls per proj tile
# constant attenuation folded into exp so the fp16 running sums G can't
# overflow (exp(s) can reach ~6e4); cancels in the softmax normalization
EBIAS = math.log(1.0 / 256.0)

# split-fp8 output projection: yn8 carries OY*y (y ~ N(0,1)-ish), wo is
# host-scaled by OW; outT carries OY*OW, divided on host.
OY = 32.0
OW = 32.0
OSCALE = OY * OW                    # 1024
# yn = 256*y out of the PV normalize, so the fused (u*s)*rb uses OY/256
YSC = OY / PSCALE

_CACHED_NC = None
_OCOPY_MIX = True   # o_sb copies alternate DVE/ACT


class _FillerQ:
    """Two priority classes of self-contained PE work: small (outproj
    groups, ~0.4us) and big (projection groups, ~2.5us). Pops follow a
    small-small-big pattern so at most ~2 small drains are ever pending
    between big groups, keeping the 3-deep pp PSUM pipeline covered."""

    def __init__(self):
        self.small = []
        self.big = []
        self._cnt = 0

    def __len__(self):
        return len(self.small) + len(self.big)

    def pop(self):
        use_big = self._cnt % 3 == 2
        self._cnt += 1
        if use_big:
            q = self.big if self.big else self.small
        else:
            q = self.small if self.small else self.big
        if q:
            q.pop(0)()


def build_program():
    nc = bacc.Bacc("TRN2", target_bir_lowering=False, debug=False)

    # input layouts are chunk/partition-major so every DMA descriptor is a
    # long contiguous run (xt: 16KB per partition per chunk)
    NCH = B * TC
    xT = nc.dram_tensor("xT", [NCH * 128, 2 * KD * 512], F8,
                        kind="ExternalInput").ap()
    wqkv = nc.dram_tensor("wqkv", [128, NDR * 2 * 3 * JL], F8,
                          kind="ExternalInput").ap()
    bqkv = nc.dram_tensor("bqkv", [3 * JL], F32, kind="ExternalInput").ap()
    wo = nc.dram_tensor("wo", [128, 3 * 2 * D_MODEL], F8,
                        kind="ExternalInput").ap()
    # outT is (dm-pair, chunk, partition)-major: each outproj pair writes one
    # [128, 1024] tile as 128 contiguous 2KB runs
    outT = nc.dram_tensor("outT", [(D_MODEL // 256) * NCH * 128, 1024], BF16,
                          kind="ExternalOutput").ap()

    xT_r = xT.rearrange("(c p) (k t) -> p c k t", p=128, k=2 * KD)

    with tile.TileContext(nc) as tc:
        with (
            tc.tile_pool(name="const", bufs=1) as const,
            tc.tile_pool(name="work", bufs=1) as work,
            tc.tile_pool(name="psum", bufs=1, space="PSUM") as psum,
        ):
            # ---- constants ----
            # Load order matters at startup: the first matmul group only
            # needs wqkv i-chunk 0 and the first xt chunk (emitted by the
            # first _emit_batch), so everything else trails them.
            wqkv_sb = const.tile([128, NDR, 2, 3 * JL], F8)
            wqkv_r = wqkv.rearrange("p (i g j) -> p i g j", i=NDR, g=2)
            nc.sync.dma_start(wqkv_sb[:, 0:3, :, :], wqkv_r[:, 0:3, :, :])
            bqkv_sb = const.tile([128, NJM], F32)
            nc.sync.dma_start(bqkv_sb[:], bqkv.rearrange("(m p) -> p m", p=128))

            def load_trailing_consts():
                for kc in range(1, 8):
                    nc.sync.dma_start(wqkv_sb[:, 3 * kc:3 * (kc + 1), :, :],
                                      wqkv_r[:, 3 * kc:3 * (kc + 1), :, :])
                nc.sync.dma_start(wo_sb[:], wo.rearrange("p (i g d) -> p i g d",
                                                         i=3, g=2))
            wo_sb = const.tile([128, 3, 2, D_MODEL], F8)

            ident = const.tile([128, 128], BF16)
            make_identity(nc, ident[:])
            ones_c = const.tile([128, 1], F32)
            nc.gpsimd.memset(ones_c[:], 1.0)
            ebias_c = const.tile([128, 1], F32)
            nc.gpsimd.memset(ebias_c[:], EBIAS)

            # masks[i][r, u] = 1.0 if u >= 128*i + r else 0  (diagonal tiles)
            masks = const.tile([128, 4, 512], BF16)
            nc.gpsimd.memset(masks[:], 1.0)
            for i in range(4):
                nc.gpsimd.affine_select(
                    out=masks[:, i, :],
                    in_=masks[:, i, :],
                    compare_op=mybir.AluOpType.is_ge,
                    fill=0.0,
                    base=-128 * i,
                    pattern=[[1, 512]],
                    channel_multiplier=-1,
                )

            filler_q = _FillerQ()
            qkvT_all = {}        # per-batch qkvT tiles (created one batch early)
            v_sb_all = {}
            for b in range(B):
                _emit_batch(nc, tc, work, psum, b,
                            xT_r, wqkv_sb, bqkv_sb, wo_sb,
                            ident, ebias_c, masks, outT, filler_q, qkvT_all,
                            v_sb_all,
                            post_first_xt=load_trailing_consts if b == 0 else None)
            while len(filler_q):
                filler_q.pop()

    nc.compile()
    return nc


def _make_proj_groups(nc, work, psum, b, xT_r, wqkv_sb, bqkv_sb, qkvT, v_sb,
                      ident, split_first_dma=False):
    """Per-(tcn, jm) projection groups as self-contained filler thunks.

    Split-fp8 DoubleRow: per k-pair m (k-tiles 2m, 2m+1; x slots 4m..4m+3
    = xh0,xl0,xh1,xl1) three DR matmuls contract both k-tiles with hi*hi,
    hi*lo and lo*hi cross terms (w packed to match on the host).
    Each chunk's group list also carries the V transposes for the chunk's
    token range, placed right after the v-column projections they read.
    """
    t0 = b * S

    def make_dma(tcn, xt):
        cidx = b * TC + tcn
        def thunk():
            if split_first_dma and tcn == 0:
                for q4 in range(4):
                    nc.sync.dma_start(
                        xt[:, 8 * q4:8 * (q4 + 1), :],
                        xT_r[:, cidx, 8 * q4:8 * (q4 + 1), :])
            else:
                nc.sync.dma_start(xt[:], xT_r[:, cidx, :, :])
        return thunk

    def make_group(tcn, jm, xt):
        def thunk():
          with nc.named_scope(f"proj.b{b}.t{tcn}"):
            js = slice(jm * 128, (jm + 1) * 128)
            ps = psum.tile([128, 512], F32, tag="pp", bufs=3)
            for m in range(KD // 2):
                rhs_abc = (
                    xt[:, 4 * m:4 * m + 2, :],          # (xh0, xl0)
                    xt[:, 4 * m:4 * m + 3:2, :],        # (xh0, xh1)
                    xt[:, 4 * m + 2:4 * m + 4, :],      # (xh1, xl1)
                )
                for c3 in range(3):
                    i = 3 * m + c3
                    nc.tensor.matmul(
                        ps[:],
                        lhsT=wqkv_sb[:, i, :, js],
                        rhs=rhs_abc[c3],
                        start=(i == 0), stop=(i == NDR - 1),
                        perf_mode=mybir.MatmulPerfMode.DoubleRow,
                    )
            # GPSIMD can't read PSUM, so drains alternate DVE / ACT
            if jm % 2 == 0:
                nc.vector.tensor_scalar_add(
                    qkvT[:, jm, tcn * 512:(tcn + 1) * 512], ps[:],
                    bqkv_sb[:, jm:jm + 1],
                )
            else:
                nc.scalar.activation(
                    qkvT[:, jm, tcn * 512:(tcn + 1) * 512], ps[:],
                    mybir.ActivationFunctionType.Identity,
                    bias=bqkv_sb[:, jm:jm + 1],
                )
        return thunk

    def make_vtr(tcn, h):
        def thunk():
          with nc.named_scope(f"vtr.b{b}.h{h}"):
            for m in range(4 * tcn, 4 * tcn + 4):
                vt_ps = psum.tile([128, 128], BF16, tag="pp", bufs=3)
                nc.tensor.transpose(
                    vt_ps[:], qkvT[:, 2 * H_PER + h, m * 128:(m + 1) * 128],
                    ident[:]
                )
                nc.vector.tensor_copy(v_sb[:, h, m, :], vt_ps[:])
        return thunk

    chunks = []
    for tcn in range(TC):
        xt = work.tile([128, 2 * KD, 512], F8, tag="xt", bufs=2, name=f"xt{b}_{tcn}")
        dma = make_dma(tcn, xt)
        groups = [make_group(tcn, jm, xt) for jm in range(5)]
        groups.append(make_vtr(tcn, 0))
        groups.append(make_group(tcn, 5, xt))
        groups.append(make_vtr(tcn, 1))
        chunks.append((dma, groups))
    return chunks


def _emit_batch(nc, tc, work, psum, b, xT_r, wqkv_sb, bqkv_sb, wo_sb,
                ident, ebias_c, masks, outT, filler_q, qkvT_all, v_sb_all,
                post_first_xt=None):
    t0 = b * S

    # ---- q/k/v projections + V transposes ----
    # Batch 0's run inline (nothing to overlap with); later batches' were
    # already emitted as fillers during the previous batch's attention —
    # flush any stragglers now (attention below reads qkvT and v_sb).
    if b == 0:
        qkvT_all[0] = work.tile([128, NJM, S], BF16, tag="qkvT", bufs=2, name="qkvT0")
        v_sb_all[0] = work.tile([128, H_PER, S // 128, 128], BF16, tag="v",
                                bufs=2, name="v0")
        chunks = _make_proj_groups(nc, work, psum, 0, xT_r, wqkv_sb, bqkv_sb,
                                   qkvT_all[0], v_sb_all[0], ident,
                                   split_first_dma=True)
        for tcn, (dma, groups) in enumerate(chunks):
            dma()
            if tcn == 0:
                post_first_xt()
            for g in groups:
                g()
    qkvT = qkvT_all[b]
    v_sb = v_sb_all[b]
    while filler_q.big:
        filler_q.pop()

    # next batch's projection groups become fillers for this batch's
    # attention; chunk tcn is enqueued at unit boundary 2*tcn so its xt DMA
    # has a full chunk of lead time
    next_chunks = None
    if b + 1 < B:
        qkvT_all[b + 1] = work.tile([128, NJM, S], BF16, tag="qkvT", bufs=2,
                                    name=f"qkvT{b + 1}")
        v_sb_all[b + 1] = work.tile([128, H_PER, S // 128, 128], BF16, tag="v",
                                    bufs=2, name=f"v{b + 1}")
        next_chunks = _make_proj_groups(nc, work, psum, b + 1, xT_r,
                                        wqkv_sb, bqkv_sb, qkvT_all[b + 1],
                                        v_sb_all[b + 1], ident)

    # ---- attention (c-major unit order) ----
    # yn8[:, h, 0/1, t] holds the hi/lo split-fp8 of OY*y for head h
    yn8 = work.tile([128, H_PER, 2, S], F8, tag="yn", bufs=2)
    pending = None

    def emit_norm(p):
      with nc.named_scope(f"norm.b{b}"):
        gs_, u_, h_, c_ = p
        cs = slice(c_ * 512, (c_ + 1) * 512)
        if len(gs_) > 1:
            nc.vector.tensor_add(gs_[0][:], gs_[0][:], gs_[1][:])
        gq = work.tile([128, 512], F16, tag="gq", bufs=2)
        nc.vector.tensor_add(gq[:], gs_[0][:, 0, :], gs_[0][:, 1, :])
        import concourse.bass_isa as bass_isa
        rb_sb = work.tile([128, 512], F32, tag="rb", bufs=2)
        nc.gpsimd.partition_all_reduce(rb_sb[:], gq[:], channels=128,
                                       reduce_op=bass_isa.ReduceOp.add)
        nc.vector.reciprocal(rb_sb[:], rb_sb[:])
        t16 = work.tile([128, 512], F16, tag="t16", bufs=2)
        nc.vector.scalar_tensor_tensor(
            t16[:], u_[:], YSC, rb_sb[:],
            op0=mybir.AluOpType.mult, op1=mybir.AluOpType.mult,
        )
        nc.scalar.copy(yn8[:, h_, 0, cs], t16[:])
        nc.vector.tensor_sub(yn8[:, h_, 1, cs], t16[:], yn8[:, h_, 0, cs])

    outT_r = outT.rearrange("(m c p) w -> p m c w", p=128, m=D_MODEL // 256)

    def make_outproj(dmp, tcn, yn8=yn8, tag="pp"):
        cidx = b * TC + tcn

        def thunk():
          with nc.named_scope(f"oproj.b{b}"):
            cs = slice(tcn * 512, (tcn + 1) * 512)
            rhs_abc = (
                yn8[:, 0, 0:2, cs],     # (yh0, yl0)
                yn8[:, 0:2, 0, cs],     # (yh0, yh1)
                yn8[:, 1, 0:2, cs],     # (yh1, yl1)
            )
            o_sb = work.tile([128, 2, 512], BF16, tag="osb", bufs=3)
            for half in range(2):
                dm = 2 * dmp + half
                ps = psum.tile([128, 512], F32, tag=tag, bufs=3)
                for i in range(3):
                    nc.tensor.matmul(
                        ps[:],
                        lhsT=wo_sb[:, i, :, dm * 128:(dm + 1) * 128],
                        rhs=rhs_abc[i],
                        start=(i == 0), stop=(i == 2),
                        perf_mode=mybir.MatmulPerfMode.DoubleRow,
                    )
                if _OCOPY_MIX and half == 1:
                    nc.scalar.copy(o_sb[:, half, :], ps[:])
                else:
                    nc.vector.tensor_copy(o_sb[:, half, :], ps[:])
            nc.sync.dma_start(outT_r[:, dmp, cidx, :], o_sb[:])
        return thunk

    n_units = H_PER * TC
    unit_idx = 0
    for c in range(TC):
        nm = 4 * (c + 1)                # valid 128-wide key tiles
        for h in range(H_PER):
            qT = qkvT[:, h, :]
            kT = qkvT[:, H_PER + h, :]
            # normalization of the previous unit goes first so its pool/DVE
            # ops are not stuck behind this unit's accumulation chain
            if pending is not None:
                emit_norm(pending)
                pending = None
            # enqueue work that just became ready at this boundary
            if unit_idx % 2 == 0:
                if next_chunks is not None and unit_idx // 2 < TC:
                    dma, groups = next_chunks[unit_idx // 2]
                    dma()
                    filler_q.big.extend(groups)
                if unit_idx >= 2:
                    cr = unit_idx // 2 - 1   # norms of (·, cr) now emitted
                    filler_q.small.extend(make_outproj(dmp, cr)
                                          for dmp in range(D_MODEL // 256))
            # The exp-sum G is accumulated in two independent partial chains
            # (DVE 2/3 of tiles, GPSIMD 1/3) so neither engine's serial chain
            # outlasts the unit; the norm that consumes them is deferred by
            # one unit, and sums both partials into one PSUM accumulator.
            # Score tiles go diagonal-first so the masked tiles' exp+mask
            # are long done when the PV chain reaches them.
            with nc.named_scope(f"att.b{b}.u{unit_idx}"):
              g_d = g_p = None
              e_pairs = [None] * (nm // 2)
              npr = nm // 2
              # diagonal pairs first so their exp+mask (DVE-queued) are long
              # done when the PV chain reaches them
              pr_order = [npr - 2, npr - 1] + list(range(npr - 2))
              # spread fillers through the ACT-paced loop so PE always has
              # independent work; aim to drain the queue by batch end
              units_left = n_units - unit_idx
              allow = min(len(filler_q), -(-len(filler_q) // units_left) + 2)
              spots = {}
              if allow:
                  for sp in np.linspace(0, npr - 1, allow).astype(int).tolist():
                      spots[sp] = spots.get(sp, 0) + 1
              for pi, pr in enumerate(pr_order):
                  for _ in range(spots.get(pi, 0)):
                      filler_q.pop()
                  s2 = psum.tile([128, 2, 512], F32, tag="s2", bufs=1)
                  for i in range(2):
                      m = 2 * pr + i
                      nc.tensor.matmul(
                          s2[:, i, :],
                          lhsT=kT[:, m * 128:(m + 1) * 128],
                          rhs=qT[:, c * 512:(c + 1) * 512],
                          start=True, stop=True,
                      )
                  e = work.tile([128, 2, 512], BF16, tag="e", bufs=9)
                  nc.scalar.activation(e[:], s2[:], mybir.ActivationFunctionType.Exp,
                                       scale=SCALE / (PSCALE * PSCALE),
                                       bias=ebias_c[:])
                  if pr >= npr - 2:      # diagonal pairs get the causal mask
                      i0 = 2 * (pr - (npr - 2))
                      nc.vector.tensor_mul(e[:], e[:], masks[:, i0:i0 + 2, :])
                  if pi % 3 == 2:
                      if g_p is None:
                          g_p = work.tile([128, 2, 512], F16, tag="gp", bufs=2)
                          nc.gpsimd.tensor_copy(g_p[:], e[:])
                      else:
                          nc.gpsimd.tensor_add(g_p[:], g_p[:], e[:])
                  else:
                      if g_d is None:
                          g_d = work.tile([128, 2, 512], F16, tag="g", bufs=2)
                          nc.vector.tensor_copy(g_d[:], e[:])
                      else:
                          nc.vector.tensor_add(g_d[:], g_d[:], e[:])
                  e_pairs[pr] = e

              u = psum.tile([128, 512], F32, tag="u", bufs=2)
              for m in range(nm):
                  nc.tensor.matmul(
                      u[:],
                      lhsT=v_sb[:, h, m, :],
                      rhs=e_pairs[m // 2][:, m % 2, :],
                      start=(m == 0), stop=(m == nm - 1),
                  )
              pending = ([g for g in (g_d, g_p) if g is not None], u, h, c)
            unit_idx += 1
    emit_norm(pending)
    # last chunk's outproj joins the queue; for the final batch the caller
    # flushes everything that remains, so those groups alternate PSUM tags
    # (s2 is idle by then) for a deeper drain pipeline in the bare tail
    tail = b == B - 1
    filler_q.small.extend(
        make_outproj(dmp, TC - 1, tag=("s2" if tail and dmp % 2 else "pp"))
        for dmp in range(D_MODEL // 256))


def _split_f8(a):
    hi = a.astype(ml_dtypes.float8_e4m3)
    lo = (a - hi.astype(np.float32)).astype(ml_dtypes.float8_e4m3)
    return hi, lo


def make_in_maps(x, Wq, bq, Wk, bk, Wv, bv, Wo, bo):
    # x: scale by SX, split hi/lo fp8, interleave k-slots (xh0,xl0,xh1,xl1,..)
    # then go chunk-major: xdr[c, p, k, t] so each DMA descriptor is the
    # 16KB contiguous (k, t) run of one partition
    xT_np = np.ascontiguousarray(x.reshape(T, D_MODEL).T) * np.float32(SX)
    xh, xl = _split_f8(xT_np)
    xs = np.stack([xh.reshape(KD, 128, T), xl.reshape(KD, 128, T)],
                  axis=1).reshape(2 * KD, 128, T)
    xdr = np.ascontiguousarray(
        xs.reshape(2 * KD, 128, B * TC, 512).transpose(2, 1, 0, 3)
    ).reshape(B * TC * 128, 2 * KD * 512)

    in_maps = []
    for c in range(N_CORES):
        sl = slice(c * JL, (c + 1) * JL)
        w = np.concatenate(
            [Wq[:, sl], Wk[:, sl], Wv[:, sl]], axis=1) * np.float32(SW)
        wh, wl = _split_f8(w)
        wh = wh.reshape(KD, 128, 3 * JL)
        wl = wl.reshape(KD, 128, 3 * JL)
        # DR instruction i groups (g=0,1); per k-pair m: A=(wh0,wh0),
        # B=(wl0,wh1), C=(wh1,wl1) matching the x-slot APs in _emit_batch
        wdr = np.empty((NDR, 2, 128, 3 * JL), ml_dtypes.float8_e4m3)
        for m in range(KD // 2):
            k0, k1 = 2 * m, 2 * m + 1
            wdr[3 * m, 0] = wh[k0]
            wdr[3 * m, 1] = wh[k0]
            wdr[3 * m + 1, 0] = wl[k0]
            wdr[3 * m + 1, 1] = wh[k1]
            wdr[3 * m + 2, 0] = wl[k1]
            wdr[3 * m + 2, 1] = wh[k1]
        bqkv_np = (np.concatenate([bq[sl], bk[sl], bv[sl]])
                   * np.float32(PSCALE)).astype(np.float32)
        # wo: same 3-term split-fp8 packing over its two 128-row k-tiles
        woh, wol = _split_f8(np.ascontiguousarray(Wo[sl, :]) * np.float32(OW))
        woh = woh.reshape(2, 128, D_MODEL)
        wol = wol.reshape(2, 128, D_MODEL)
        wodr = np.empty((3, 2, 128, D_MODEL), ml_dtypes.float8_e4m3)
        wodr[0, 0] = woh[0]
        wodr[0, 1] = woh[0]
        wodr[1, 0] = wol[0]
        wodr[1, 1] = woh[1]
        wodr[2, 0] = wol[1]
        wodr[2, 1] = woh[1]
        in_maps.append({
            "xT": xdr,
            "wqkv": np.ascontiguousarray(
                wdr.transpose(2, 0, 1, 3)).reshape(128, NDR * 2 * 3 * JL),
            "bqkv": bqkv_np,
            "wo": np.ascontiguousarray(
                wodr.transpose(2, 0, 1, 3)).reshape(128, 3 * 2 * D_MODEL),
        })
    return in_maps


def kernel(x, Wq, bq, Wk, bk, Wv, bv, Wo, bo):
    global _CACHED_NC
    x, Wq, bq, Wk, bk, Wv, bv, Wo, bo = [
        np.asarray(a, np.float32) for a in (x, Wq, bq, Wk, bk, Wv, bv, Wo, bo)
    ]
    if _CACHED_NC is None:
        _CACHED_NC = build_program()
    nc = _CACHED_NC

    in_maps = make_in_maps(x, Wq, bq, Wk, bk, Wv, bv, Wo, bo)
    res = run_bass_kernel_spmd(nc, in_maps, core_ids=list(range(N_CORES)))

    acc = res.results[0]["outT"].astype(np.float32)
    for c in range(1, N_CORES):
        acc += res.results[c]["outT"].astype(np.float32)
    # outT[dmp, c, p, slot, t]: d = (2*dmp+slot)*128+p, token = c*512+t
    outTf = acc.reshape(D_MODEL // 256, B * TC, 128, 2, 512).transpose(
        0, 3, 2, 1, 4).reshape(D_MODEL, T)
    out = outTf.T * np.float32(1.0 / OSCALE) + bo[None, :]
    return np.ascontiguousarray(out.reshape(B, S, D_MODEL), dtype=np.float32)


# ---------------------------------------------------------------- dev tools

def _np_partial_reference(inputs, core):
    """fp32 numpy partial output for one core's heads (no bo)."""
    x = np.asarray(inputs["x"], np.float32).reshape(T, D_MODEL)
    sl = slice(core * JL, (core + 1) * JL)
    q = x @ np.asarray(inputs["Wq"])[:, sl] + np.asarray(inputs["bq"])[sl]
    k = x @ np.asarray(inputs["Wk"])[:, sl] + np.asarray(inputs["bk"])[sl]
    v = x @ np.asarray(inputs["Wv"])[:, sl] + np.asarray(inputs["bv"])[sl]
    y = np.zeros((T, JL), np.float32)
    for b in range(B):
        tb = slice(b * S, (b + 1) * S)
        for h in range(H_PER):
            js = slice(h * HEAD_DIM, (h + 1) * HEAD_DIM)
            qh, kh, vh = q[tb, js], k[tb, js], v[tb, js]
            s = (qh @ kh.T) * SCALE
            mask = np.triu(np.ones((S, S), bool), k=1)
            s[mask] = -np.inf
            s -= s.max(axis=1, keepdims=True)
            p = np.exp(s)
            p /= p.sum(axis=1, keepdims=True)
            y[tb, js] = p @ vh
    return (y @ np.asarray(inputs["Wo"])[sl, :]).T  # [D, T]


def _simulate_core0():
    import reference
    from concourse.bass_interp import CoreSim

    inputs = {k: np.asarray(v) for k, v in reference.setup_inputs().items()}
    nc = build_program()
    in_map = make_in_maps(**inputs)[0]

    sim = CoreSim(nc)
    for name, arr in in_map.items():
        sim.tensor(name)[:] = arr
    sim.simulate(check_with_hw=False)
    got = (np.asarray(sim.tensor("outT"), np.float32)
           .reshape(D_MODEL // 256, B * TC, 128, 2, 512)
           .transpose(0, 3, 2, 1, 4).reshape(D_MODEL, T) / OSCALE)

    want = _np_partial_reference(inputs, 0)
    denom = np.abs(want).max()
    err = np.abs(got - want).max() / denom
    print(f"sim core0 partial: max={np.abs(got).max():.4f} "
          f"absmax_err={np.abs(got - want).max():.5f} rel={err:.5f}")


if __name__ == "__main__":
    import sys
    if "--sim" in sys.argv:
        _simulate_core0()
    else:
        nc = build_program()
        n_inst = sum(len(bb.instructions) for bb in nc.m.functions[0].blocks)
        print(f"built: {n_inst} instructions")

